# revision 4
# baseline (speedup 1.0000x reference)
"""Trainium2 Bass kernel for CellSizePredictor (v5: batch-major fp8 DR).

reference:
    average = x[:, :n]; numbers = x[:, n:]
    o = numbers * average**alpha
    out = o @ A + einsum('bi,ij,bj->b', o, B, o) + C

Math (host): mu = column-mean of o, d = o - mu, S = (B+B^T)/2,
U = triu(B+B^T,1)+diag(B):
    out = [d^T U d]_device + [d @ (A + 2 S mu) + C + mu@A + mu^T S mu]_host-linear
The linear vector rides in as an input and is added on device at the end.

Device (data-parallel, batch shard 8192 rows/core, 64 chunks of 128):
  * Batch-major: stationary = d8 fp8e4 feature-major pairs [128i,2,128b]
    (DoubleRow, 0.5 cyc/col), moving = resident U8 pairs -> z [128b, j]
    in PSUM. Triangular trim: 6 MMs / 2560 cols per chunk.
  * ACT casts z -> fp16 SBUF (its only job; PSUM-read engine).
  * DVE: one scalar_tensor_tensor per chunk, fp16 2x mode:
    dummy = (z16 * 1) * d16, accum_out = per-row sum = quad_b. No tree,
    no ones-matmuls; PE never waits on the epilogue (keeps HAM warm).
  * d16 DMA on sync queue, d8 on gpsimd queue (384KB/chunk > one
    queue's practical bandwidth at the 1.2us/chunk PE pace).
"""
import sys

for _p in ("/opt/trn_rl_repo",):
    if _p not in sys.path:
        sys.path.append(_p)

import numpy as np
import ml_dtypes
from contextlib import ExitStack

import concourse.bass as bass
import concourse.tile as tile
from concourse import bacc, mybir
from concourse.bass_utils import run_bass_kernel_spmd

dt = mybir.dt
F32 = dt.float32
F16 = dt.float16
F8 = dt.float8e4
NP_F8 = ml_dtypes.float8_e4m3
DR = mybir.MatmulPerfMode.DoubleRow

N_CORES = 8
BATCH = 65536
N = 1024
SHARD = BATCH // N_CORES          # 8192
N_IC = N // 128                   # 8 k-subtiles
SUP = 1024                        # rows per "n_sup" unit (host API compat)
N_SUP = SHARD // SUP              # 8
CPS = SUP // 128                  # chunks per sup = 8
N_WARM = 2


def _build(n_sup: int):
    nc = bacc.Bacc("TRN2", target_bir_lowering=False, debug=False)

    rows = n_sup * SUP
    chunks = rows // 128
    d16_d = nc.dram_tensor("d16", [128, chunks * N], F16,
                           kind="ExternalInput").ap()
    d8_d = nc.dram_tensor("d8", [128, chunks * N], F8,
                          kind="ExternalInput").ap()
    u8_d = nc.dram_tensor("u8", [128, N_IC * N], F8,
                          kind="ExternalInput").ap()
    lin_d = nc.dram_tensor("lin", [128, chunks], F32,
                           kind="ExternalInput").ap()
    out_d = nc.dram_tensor("out", [128, chunks], F32,
                           kind="ExternalOutput").ap()

    with tile.TileContext(nc) as tc, ExitStack() as ctx:
        consts = ctx.enter_context(tc.tile_pool(name="consts", bufs=1))
        d16p = ctx.enter_context(tc.tile_pool(name="d16p", bufs=12))
        d8p = ctx.enter_context(tc.tile_pool(name="d8p", bufs=12))
        dump = ctx.enter_context(tc.tile_pool(name="dump", bufs=4))
        ps_z = ctx.enter_context(tc.tile_pool(name="ps_z", bufs=3, space="PSUM"))
        ps_w = ctx.enter_context(tc.tile_pool(name="ps_w", bufs=1, space="PSUM"))

        u8_sb = consts.tile([128, N_IC * N], F8)
        u8_3d = u8_sb[:].rearrange("p (t f) -> p t f", t=N_IC)
        lin_sb = consts.tile([128, chunks], F32)
        colbuf = consts.tile([128, chunks], F32)
        res_sb = consts.tile([128, chunks], F32)
        ones_f = consts.tile([128, 1], F32)
        nc.vector.memset(ones_f[:], 1.0)
        warm_h = consts.tile([128, 512], F16)
        nc.vector.memset(warm_h[:], 0.0)
        ones_h = consts.tile([128, 1], F16)
        nc.vector.tensor_copy(ones_h[:], ones_f[:])

        nc.sync.dma_start(u8_sb[:], u8_d[:, :])
        nc.sync.dma_start(lin_sb[:], lin_d)

        # PE warmup overlapping initial DMA
        p_warm = ps_w.tile([1, 512], F32, tag="pwarm")
        for w in range(N_WARM):
            nc.tensor.matmul(p_warm[:], ones_h[:], warm_h[:],
                             start=(w == 0), stop=(w == N_WARM - 1))

        for c in range(chunks):
            csl = slice(c * N, (c + 1) * N)
            d16t = d16p.tile([128, N], F16, tag="d16")
            nc.scalar.dma_start(d16t[:], d16_d[:, csl])
            d8t = d8p.tile([128, N], F8, tag="d8")
            if c < 4:
                # HWDGE rings ramp much faster than SWDGE; keep the pipeline
                # head on the scalar ring and steady state on the (otherwise
                # idle) sync ring
                nc.scalar.dma_start(d8t[:], d8_d[:, csl])
            else:
                nc.gpsimd.dma_start(d8t[:], d8_d[:, csl])
            d8_3d = d8t[:].rearrange("p (t m) -> p t m", t=N_IC)

            p_z = ps_z.tile([128, N], F32, tag="pz")
            for q in range(4):
                lhsT = d8_3d[:, 2 * q:2 * q + 2, :]
                for h in range(2):
                    j0 = max(512 * h, 256 * q)
                    j1 = 512 * (h + 1)
                    if j0 >= j1:
                        continue
                    nc.tensor.matmul(
                        p_z[:, j0:j1],
                        lhsT,
                        u8_3d[:, 2 * q:2 * q + 2, j0:j1],
                        start=(q == 0),
                        stop=(h == 0 and q == 1) or (h == 1 and q == 3),
                        perf_mode=DR,
                        skip_group_check=True,
                    )
            psTd = dump.tile([128, N], F16, tag="psTd")
            nc.vector.scalar_tensor_tensor(
                out=psTd[:],
                in0=p_z[:],
                scalar=1.0,
                in1=d16t[:],
                op0=mybir.AluOpType.mult,
                op1=mybir.AluOpType.mult,
                accum_out=colbuf[:, c:c + 1],
            )

        nc.vector.tensor_add(res_sb[:], colbuf[:], lin_sb[:])
        nc.sync.dma_start(out_d[:, :], res_sb[:])

    nc.compile()
    return nc


_CACHE: dict = {}


def _get_program(n_sup: int):
    if n_sup not in _CACHE:
        _CACHE[n_sup] = _build(n_sup)
    return _CACHE[n_sup]


def kernel(x, A, B, C, alpha, _n_sup=N_SUP, _trace=False):
    x = np.asarray(x, dtype=np.float32)
    A = np.asarray(A, dtype=np.float32)
    B = np.asarray(B, dtype=np.float32)
    C = np.asarray(C, dtype=np.float32).reshape(-1)
    alpha = np.asarray(alpha, dtype=np.float32)
    assert x.shape == (BATCH, 2 * N), x.shape

    if not np.all(alpha == 1.0):
        o = x[:, N:] * np.power(x[:, :N], alpha[None, :])
        return (o @ A + np.einsum("bi,ij,bj->b", o, B, o) + C[0]).astype(
            np.float32
        )

    nc = _get_program(_n_sup)

    o = x[:, N:] * x[:, :N]
    mu = o.mean(axis=0).astype(np.float32)
    d = o - mu
    Bs = B + B.T
    S = 0.5 * Bs
    U = np.triu(Bs, 1) + np.diag(np.diag(B))
    U8 = U.astype(NP_F8)
    Ap = A + 2.0 * (S @ mu)
    Cpp = float(C[0]) + float(mu @ A) + float(mu @ (S @ mu))
    linear = (d @ Ap + Cpp).astype(np.float32)     # [BATCH]

    U8L = np.ascontiguousarray(
        U8.reshape(N_IC, 128, N).transpose(1, 0, 2)
    ).reshape(128, N_IC * N)

    rows = _n_sup * SUP
    chunks = rows // 128
    d16 = d.astype(np.float16)
    d8 = d.astype(NP_F8)
    in_maps = []
    for c in range(N_CORES):
        sl = slice(c * SHARD, c * SHARD + rows)
        # batch-major [128 b, (chunk, j)]
        dbm = np.ascontiguousarray(
            d16[sl].reshape(chunks, 128, N).transpose(1, 0, 2)
        ).reshape(128, chunks * N)
        # feature-major pairs [128 i, (chunk, t, m)]
        dfm = np.ascontiguousarray(
            d8[sl].reshape(chunks, 128, N_IC, 128).transpose(3, 0, 2, 1)
        ).reshape(128, chunks * N)
        linc = np.ascontiguousarray(
            linear[sl].reshape(chunks, 128).T
        ).astype(np.float32)
        in_maps.append({"d16": dbm, "d8": dfm, "u8": U8L, "lin": linc})
    res = run_bass_kernel_spmd(
        nc, in_maps, list(range(N_CORES)), trace=_trace
    )
    if _trace:
        kernel._last_results = res
    out = np.empty(N_CORES * rows, dtype=np.float32)
    for c in range(N_CORES):
        out[c * rows: (c + 1) * rows] = np.ascontiguousarray(
            res.results[c]["out"].T
        ).reshape(-1)
    if rows == SHARD:
        return out
    full = np.zeros(BATCH, dtype=np.float32)
    for c in range(N_CORES):
        full[c * SHARD: c * SHARD + rows] = out[c * rows: (c + 1) * rows]
    return full
